# revision 56
# baseline (speedup 1.0000x reference)
"""HarmonyGenerator Trainium2 kernel.

Math: the reference's 3x3 conv on [T,1,1,D] degenerates to a 3-tap conv along
the feature axis (only the kernel's middle row touches data).  Conv and the
three linear heads are both linear, so the conv folds into the head weights
(W' = 3-tap correlation of W along K) and the constant context-embedding rows
plus conv bias fold into the output bias.  The device work is one GEMM:

    out[2048, 168] = [melody | lyrics][2048, 50681] @ W'[50681, 168] + bias

Sharding: K (feature) axis split 8 ways, 6400 rows per core (zero padded).
Each core reads 1/8 of x AND 1/8 of W and produces a partial [168, 2048];
partials are summed on the host during the gather/unshard step.

Device schedule: everything rides the two HWDGE rings (sync+scalar),
byte-balanced, ~205 GB/s each / ~410 GB/s aggregate (the per-core DMA
ceiling; measured identical whatever the ring/queue mix).  The rings allow
only ~2 in-flight DMA instructions each, so instruction sizes are kept
homogeneous: 1-ktile x chunks while the PE is in its 1.2 GHz p-state ramp,
1 MB chunks after, and W in two small primers plus two ~0.8 MB bulks right
behind the last singles (mid-stream W wedges and the gpsimd SWDGE path both
measured several us worse).  A warm-up block of dummy matmuls covers the
~12us DMA cold-start so the HAM utilization monitor grants full PE clock
just as real work begins.  The PE stream is grouped: per 4-ktile group, 16
mel matmuls back-to-back then 8 chord+beat pairs (tile_position column
packing), so the ~95ns LDWEIGHTS stall on stationary-shape transitions is
paid twice per group instead of twice per k-tile.  The final group staggers
per-bank evictions (vector/scalar copies, stores alternating rings) under
the last matmuls; the chord+beat PSUM banks dump raw to a scratch dram
tensor (one clean 2KB-line DMA each) and the host assembles rows 128:168.

Measured: 109.8us (prior session baseline) -> ~96us.
"""

import os
import numpy as np

import concourse.bacc as bacc
import concourse.mybir as mybir
from concourse.tile import TileContext
from concourse.bass_utils import run_bass_kernel_spmd

# Problem shapes (hardcoded per contract)
T = 2048               # steps = length * 128
D_IN = 50937           # 256 ctx + 256 melody/vel + 50425 lyrics
K_GEMM = 50681         # melody(256) + lyrics(50425) features in the GEMM
N_OUT = 168            # 24 chord + 16 beat + 128 mel
N_CORES = 8
K_PER = 6400           # per-core K (8*6400 = 51200 >= 50681, zero padded)
KT = K_PER // 128      # 50 k-tiles per core
TB = 512               # t-block (max fp32 moving dim / PSUM bank)
NTB = T // TB          # 4

_NC = None
LAST_RESULT = None     # BassKernelResults of the most recent run (for test.py)

# Matmul input dtype: fp16 (half the x DMA traffic, ~4e-4 rel err) or
# f32r (fp32 bytes, FP22 multiply, ~2e-4 rel err).
DTYPE = os.environ.get("HARMONY_DTYPE", "fp16")

# x chunk schedule in k-tiles: small head chunks to fill the pipeline fast,
# 2-ktile (1 MB) steady chunks, small tail chunks so the final PE group's
# dependency lands early.  28 chunks, alternating sync/scalar rings.
X_SCHED = [1] * 12 + [2] * 18 + [1, 1]
assert sum(X_SCHED) == KT
# All of W lands in the first ~12us: two small primers, then two bulk chunks
# right behind the head x chunks.  Mid-stream W insertions repeatedly
# measured worse (they wedge the ~2-deep in-flight DMA pipeline and the PE
# has no slack vs DMA); instead the PE start is delayed past the W+ramp
# deficit via a long warmup, after which supply stays ahead for good.
# (The gpsimd SWDGE path is worse still: it steals from the shared
# ~410 GB/s cap exactly during the critical ramp.)
W_SLICES = [(0, 6), (6, 12), (12, 31), (31, KT)]
GROUPS = [(4 * g, min(4 * g + 4, KT)) for g in range(13)]


def _in_dt():
    return mybir.dt.float16 if DTYPE == "fp16" else mybir.dt.float32r


def _np_in_dt():
    return np.float16 if DTYPE == "fp16" else np.float32


def _build_nc():
    f32 = mybir.dt.float32
    fin = _in_dt()
    nc = bacc.Bacc()
    xt = nc.dram_tensor("xt", [K_PER, T], fin, kind="ExternalInput")
    w = nc.dram_tensor("w", [128, KT * N_OUT], fin, kind="ExternalInput")
    out = nc.dram_tensor("out", [128, T], f32, kind="ExternalOutput")
    # raw dumps of the two chord+beat PSUM banks (40 live rows at partition
    # offsets 0 and 64 each); host assembles rows 128:168 from these
    out2 = nc.dram_tensor("out2", [2, 104, TB], f32, kind="ExternalOutput")

    with TileContext(nc) as tc:
        with (
            tc.tile_pool(name="wp", bufs=1) as wp,
            tc.tile_pool(name="xp", bufs=10) as xp,
            tc.tile_pool(name="op", bufs=1) as op,
            tc.tile_pool(name="ps", bufs=1, space="PSUM") as ps,
        ):
            # HAM warm-up: the PE p-state gate holds matmuls at 1.2 GHz until
            # ~3us of sustained activity.  Burn the DMA-fill window (no real
            # operands on chip yet) on dummy matmuls so real MMs start at
            # 2.4 GHz.  Scratch PSUM bank; results never read.
            dm = wp.tile([128, TB], fin, tag="warm", name="warmup")
            nc.vector.memset(dm[:], 0.0)
            ps_warm = ps.tile([128, TB], f32, tag="warm_ps", name="ps_warm")

            def warm_mm(n):
                for _ in range(n):
                    nc.tensor.matmul(ps_warm[:], dm[:, 0:128], dm[:], start=True, stop=True)

            warm_mm(13)

            # W chunk tiles: [128, n*168] column blocks of the packed w
            # tensor.  The primers are loaded before any x so the first
            # matmuls never wait on W.
            w_of = {}

            def emit_w(s, ring):
                k0, k1 = W_SLICES[s]
                wt = wp.tile([128, (k1 - k0) * N_OUT], fin, tag=f"w{s}", name=f"w{s}")
                ring.dma_start(wt[:], w[:, k0 * N_OUT:k1 * N_OUT])
                for kt in range(k0, k1):
                    w_of[kt] = (wt, (kt - k0) * N_OUT)

            # 4KB pre-warm DMA on each HWDGE ring: absorbs the ~1.6us
            # descriptor-pipeline cold-start so the real W/x chunks stream hot
            dpre = wp.tile([128, 32], fin, tag="pre", name="prewarm")
            nc.sync.dma_start(dpre[:, 0:16], xt[0:128, 0:16])
            nc.scalar.dma_start(dpre[:, 16:32], xt[0:128, 16:32])

            emit_w(0, nc.sync)
            emit_w(1, nc.scalar)

            # Persistent accumulators: 4 mel banks + 2 shared cb banks.  Each
            # cb bank holds two t-blocks' [40, TB] outputs col-tiled into
            # partitions 0:40 and 64:104 (concurrent matmuls via tile_position).
            psm = [ps.tile([128, TB], f32, tag=f"m{t}", name=f"psm{t}") for t in range(NTB)]
            psc = [ps.tile([128, TB], f32, tag=f"c{p}", name=f"psc{p}") for p in range(NTB // 2)]

            # x chunk bookkeeping: kt -> (tile, col offset); chunk c covers
            # kts [cstart[c], cstart[c]+X_SCHED[c])
            x_of = {}
            cstart = []
            k = 0
            for xn in X_SCHED:
                cstart.append(k)
                k += xn

            def emit_x(c):
                xn = X_SCHED[c]
                k0 = cstart[c]
                ring = nc.sync if c % 2 == 0 else nc.scalar
                x_tile = xp.tile([128, xn * T], fin, tag="x", name=f"x{c}")
                if xn == 1:
                    ring.dma_start(x_tile[:], xt[k0 * 128:(k0 + 1) * 128, :])
                else:
                    ring.dma_start(
                        x_tile[:].rearrange("p (a t) -> p a t", a=xn),
                        xt[k0 * 128:(k0 + xn) * 128, :].rearrange(
                            "(a p) t -> p a t", p=128
                        ),
                    )
                for a in range(xn):
                    x_of[k0 + a] = (x_tile, a * T)

            def lhs(kt):
                wt, j = w_of[kt]
                return wt[:, j:j + 128], wt[:, j + 128:j + N_OUT]

            def rhs(kt, t):
                x_tile, off = x_of[kt]
                return x_tile[:, off + t * TB:off + (t + 1) * TB]

            def mel_mm(kt, t):
                nc.tensor.matmul(psm[t][:], lhs(kt)[0], rhs(kt, t),
                                 start=kt == 0, stop=kt == KT - 1)

            def cb_pair(kt, p):
                # two concurrent 40-col matmuls in distinct col groups
                lhs_c = lhs(kt)[1]
                first, last = kt == 0, kt == KT - 1
                nc.tensor.matmul(psc[p][0:40, :], lhs_c, rhs(kt, 2 * p),
                                 start=first, stop=last, tile_position=(0, 0))
                nc.tensor.matmul(psc[p][64:104, :], lhs_c, rhs(kt, 2 * p + 1),
                                 start=first, stop=last, tile_position=(0, 64))

            # Evictions: mel bank t -> out rows 0:128, cb bank p -> rows
            # 128:168 for t-blocks 2p/2p+1.  Copies split across vector (mel)
            # and gpsimd (cb) so the tail pipeline isn't serialized on DVE.
            def evict_mel(t):
                o1 = op.tile([128, TB], f32, tag=f"o{t}", name=f"o{t}")
                if t % 2 == 0:
                    nc.vector.tensor_copy(o1[:], psm[t][:])
                else:
                    nc.scalar.copy(o1[:], psm[t][:])
                ring = nc.sync if t % 2 == 0 else nc.scalar
                ring.dma_start(out[0:128, t * TB:(t + 1) * TB], o1[:])

            def evict_cb(p):
                o2 = op.tile([104, TB], f32, tag=f"oc{p}", name=f"oc{p}")
                if p % 2 == 0:
                    nc.vector.tensor_copy(o2[:], psc[p][0:104, :])
                else:
                    nc.scalar.copy(o2[:], psc[p][0:104, :])
                ring = nc.sync if p % 2 == 0 else nc.scalar
                ring.dma_start(out2[p], o2[:])

            for g, (k0, k1) in enumerate(GROUPS):
                # DMAs for this group's chunks: single-ktile chunks through
                # kt11 (fine-grained arrivals match the half-rate early PE),
                # then 1MB chunks; the W bulks ride after the last singles.
                if g < 3:
                    for c in range(4 * g, 4 * g + 4):
                        emit_x(c)
                    if g == 2:
                        emit_w(2, nc.sync)
                        emit_w(3, nc.scalar)
                else:
                    emit_x(2 * g + 6)
                    emit_x(2 * g + 7)

                if g < len(GROUPS) - 1:
                    # grouped steady state: all mel MMs back-to-back (LDWEIGHTS
                    # pipelined), then the cb pairs; 2 stationary-shape
                    # transitions per group instead of 2 per k-tile.
                    for kt in range(k0, k1):
                        for t in range(NTB):
                            mel_mm(kt, t)
                        # group 0: the supply of x chunks c2/c3 trails the PE
                        # by several us; DMA-independent warm matmuls bridge
                        # the gap at kt granularity so the PE stream stays
                        # continuous and the HAM clock grant isn't revoked.
                        if g == 0 and k0 + 1 <= kt <= k0 + 2:
                            warm_mm(4)
                    for kt in range(k0, k1):
                        cb_pair(kt, 0)
                        cb_pair(kt, 1)
                    # early groups: DMA-independent filler matmuls keep the PE
                    # active through any x-starvation gap so the HAM
                    # utilization monitor grants full clock ASAP.
                    if g < 2:
                        warm_mm(1)
                else:
                    # final group: bank-major order so evictions start while
                    # the remaining banks' matmuls still run; cb first so the
                    # kernel ends on the cheap single-DMA mel stores.
                    for p in range(NTB // 2):
                        for kt in range(k0, k1):
                            cb_pair(kt, p)
                        evict_cb(p)
                    for t in range(NTB):
                        for kt in range(k0, k1):
                            mel_mm(kt, t)
                        evict_mel(t)
    return nc


def _get_nc():
    global _NC
    if _NC is None:
        _NC = _build_nc()
        if not _NC.is_finalized():
            _NC.finalize()
    return _NC


def kernel(**inputs):
    global LAST_RESULT
    melody = np.ascontiguousarray(np.asarray(inputs["melody_tensor"], dtype=np.float32))
    lyrics = np.ascontiguousarray(np.asarray(inputs["lyrics_tensor"], dtype=np.float32))
    emb = np.asarray(inputs["emb"], dtype=np.float32)
    conv_w = np.asarray(inputs["conv_w"], dtype=np.float32)
    conv_b = np.asarray(inputs["conv_b"], dtype=np.float32)
    w_chord = np.asarray(inputs["w_chord"], dtype=np.float32)
    w_beat = np.asarray(inputs["w_beat"], dtype=np.float32)
    w_mel = np.asarray(inputs["w_mel"], dtype=np.float32)
    b_heads = np.concatenate([
        np.asarray(inputs["b_chord"], dtype=np.float32),
        np.asarray(inputs["b_beat"], dtype=np.float32),
        np.asarray(inputs["b_mel"], dtype=np.float32),
    ])
    genre = int(np.asarray(inputs["genre"]).reshape(-1)[0])
    tempo = int(np.asarray(inputs["tempo"]).reshape(-1)[0])
    key_sig = int(np.asarray(inputs["key_sig"]).reshape(-1)[0])

    # Fold conv into head weights: W'[e] = k0*W[e+1] + k1*W[e] + k2*W[e-1]
    W = np.concatenate([w_chord, w_beat, w_mel], axis=1)  # [50937, 168]
    k0, k1, k2 = (float(v) for v in conv_w[0, 0, 1, :])
    Wp = k1 * W
    Wp[:-1] += k0 * W[1:]
    Wp[1:] += k2 * W[:-1]

    # Bias: head biases + conv bias * colsum(W) + context-embedding term
    ids = [genre, 10 + tempo, 20 + key_sig, 34]
    ctx = emb[ids].sum(axis=0).astype(np.float64)  # [256]
    bias = (
        b_heads.astype(np.float64)
        + float(conv_b[0]) * W.sum(axis=0, dtype=np.float64)
        + ctx @ Wp[0:256].astype(np.float64)
    )  # [168]

    # Device operands: xT [51200, 2048] (zero padded), W' rows 256.. packed
    np_dt = _np_in_dt()
    K_PAD = N_CORES * K_PER
    XT = np.zeros((K_PAD, T), np_dt)
    XT[0:256] = melody.T
    XT[256:K_GEMM] = lyrics.T
    Wg = np.zeros((K_PAD, N_OUT), np_dt)
    Wg[0:K_GEMM] = Wp[256:]

    in_maps = []
    for c in range(N_CORES):
        wc = (
            Wg[c * K_PER:(c + 1) * K_PER]
            .reshape(KT, 128, N_OUT)
            .transpose(1, 0, 2)
            .reshape(128, KT * N_OUT)
        )
        in_maps.append({
            "xt": XT[c * K_PER:(c + 1) * K_PER],
            "w": np.ascontiguousarray(wc),
        })

    trace = bool(os.environ.get("HARMONY_TRACE"))
    res = run_bass_kernel_spmd(_get_nc(), in_maps, core_ids=list(range(N_CORES)), trace=trace)
    LAST_RESULT = res

    acc = np.zeros((N_OUT, T), np.float64)
    for r in res.results:
        acc[0:128] += r["out"]
        o2 = r["out2"]
        for p in range(2):
            acc[128:N_OUT, 2 * p * TB:(2 * p + 1) * TB] += o2[p][0:40]
            acc[128:N_OUT, (2 * p + 1) * TB:(2 * p + 2) * TB] += o2[p][64:104]
    out = (acc + bias[:, None]).T
    return np.ascontiguousarray(out.astype(np.float32))


# revision 57
# speedup vs baseline: 1.0973x; 1.0973x over previous
"""HarmonyGenerator Trainium2 kernel.

Math: the reference's 3x3 conv on [T,1,1,D] degenerates to a 3-tap conv along
the feature axis (only the kernel's middle row touches data).  Conv and the
three linear heads are both linear, so the conv folds into the head weights
(W' = 3-tap correlation of W along K) and the constant context-embedding rows
plus conv bias fold into the output bias.  The device work is one GEMM:

    out[2048, 168] = [melody | lyrics][2048, 50681] @ W'[50681, 168] + bias

Sharding: K (feature) axis split 8 ways, 6400 rows per core (zero padded).
Each core reads 1/8 of x AND 1/8 of W and produces a partial [168, 2048];
partials are summed on the host during the gather/unshard step.

Device schedule: everything rides the two HWDGE rings (sync+scalar),
byte-balanced, ~205 GB/s each / ~410 GB/s aggregate (the per-core DMA
ceiling; measured identical whatever the ring/queue mix).  The rings allow
only ~2 in-flight DMA instructions each, so instruction sizes are kept
homogeneous: 1-ktile x chunks while the PE is in its 1.2 GHz p-state ramp,
1 MB chunks after, and W in two small primers plus two ~0.8 MB bulks right
behind the last singles (mid-stream W wedges and the gpsimd SWDGE path both
measured several us worse).  A warm-up block of dummy matmuls covers the
~12us DMA cold-start so the HAM utilization monitor grants full PE clock
just as real work begins.  The PE stream is grouped: per 4-ktile group, 16
mel matmuls back-to-back then 8 chord+beat pairs (tile_position column
packing), so the ~95ns LDWEIGHTS stall on stationary-shape transitions is
paid twice per group instead of twice per k-tile.  The final group staggers
per-bank evictions (vector/scalar copies, stores alternating rings) under
the last matmuls; the chord+beat PSUM banks dump raw to a scratch dram
tensor (one clean 2KB-line DMA each) and the host assembles rows 128:168.

Measured: 109.8us (prior session baseline) -> ~96us.
"""

import os
import numpy as np

import concourse.bacc as bacc
import concourse.mybir as mybir
from concourse.tile import TileContext
from concourse.bass_utils import run_bass_kernel_spmd

# Problem shapes (hardcoded per contract)
T = 2048               # steps = length * 128
D_IN = 50937           # 256 ctx + 256 melody/vel + 50425 lyrics
K_GEMM = 50681         # melody(256) + lyrics(50425) features in the GEMM
N_OUT = 168            # 24 chord + 16 beat + 128 mel
N_CORES = 8
K_PER = 6400           # per-core K (8*6400 = 51200 >= 50681, zero padded)
KT = K_PER // 128      # 50 k-tiles per core
TB = 512               # t-block (max fp32 moving dim / PSUM bank)
NTB = T // TB          # 4

_NC = None
LAST_RESULT = None     # BassKernelResults of the most recent run (for test.py)

# Matmul input dtype: fp16 (half the x DMA traffic, ~4e-4 rel err) or
# f32r (fp32 bytes, FP22 multiply, ~2e-4 rel err).
DTYPE = os.environ.get("HARMONY_DTYPE", "fp16")

# x chunk schedule in k-tiles: small head chunks to fill the pipeline fast,
# 2-ktile (1 MB) steady chunks, small tail chunks so the final PE group's
# dependency lands early.  28 chunks, alternating sync/scalar rings.
X_SCHED = [1] * 12 + [2] * 18 + [1, 1]
assert sum(X_SCHED) == KT
# All of W lands in the first ~12us: two small primers, then two bulk chunks
# right behind the head x chunks.  Mid-stream W insertions repeatedly
# measured worse (they wedge the ~2-deep in-flight DMA pipeline and the PE
# has no slack vs DMA); instead the PE start is delayed past the W+ramp
# deficit via a long warmup, after which supply stays ahead for good.
# (The gpsimd SWDGE path is worse still: it steals from the shared
# ~410 GB/s cap exactly during the critical ramp.)
W_SLICES = [(0, 6), (6, 12), (12, 31), (31, KT)]
GROUPS = [(4 * g, min(4 * g + 4, KT)) for g in range(13)]


def _in_dt():
    return mybir.dt.float16 if DTYPE == "fp16" else mybir.dt.float32r


def _np_in_dt():
    return np.float16 if DTYPE == "fp16" else np.float32


def _build_nc():
    f32 = mybir.dt.float32
    fin = _in_dt()
    nc = bacc.Bacc()
    xt = nc.dram_tensor("xt", [K_PER, T], fin, kind="ExternalInput")
    w = nc.dram_tensor("w", [128, KT * N_OUT], fin, kind="ExternalInput")
    out = nc.dram_tensor("out", [128, T], f32, kind="ExternalOutput")
    # raw dumps of the two chord+beat PSUM banks (40 live rows at partition
    # offsets 0 and 64 each); host assembles rows 128:168 from these
    out2 = nc.dram_tensor("out2", [2, 104, TB], f32, kind="ExternalOutput")

    with TileContext(nc) as tc:
        with (
            tc.tile_pool(name="wp", bufs=1) as wp,
            tc.tile_pool(name="xp", bufs=10) as xp,
            tc.tile_pool(name="op", bufs=1) as op,
            tc.tile_pool(name="ps", bufs=1, space="PSUM") as ps,
        ):
            # HAM warm-up: the PE p-state gate holds matmuls at 1.2 GHz until
            # ~3us of sustained activity.  Burn the DMA-fill window (no real
            # operands on chip yet) on dummy matmuls so real MMs start at
            # 2.4 GHz.  Scratch PSUM bank; results never read.
            dm = wp.tile([128, TB], fin, tag="warm", name="warmup")
            nc.vector.memset(dm[:], 0.0)
            ps_warm = ps.tile([128, TB], f32, tag="warm_ps", name="ps_warm")

            def warm_mm(n):
                for _ in range(n):
                    nc.tensor.matmul(ps_warm[:], dm[:, 0:128], dm[:], start=True, stop=True)

            warm_mm(13)

            # W chunk tiles: [128, n*168] column blocks of the packed w
            # tensor.  The primers are loaded before any x so the first
            # matmuls never wait on W.
            w_of = {}

            def emit_w(s, ring):
                k0, k1 = W_SLICES[s]
                wt = wp.tile([128, (k1 - k0) * N_OUT], fin, tag=f"w{s}", name=f"w{s}")
                ring.dma_start(wt[:], w[:, k0 * N_OUT:k1 * N_OUT])
                for kt in range(k0, k1):
                    w_of[kt] = (wt, (kt - k0) * N_OUT)

            emit_w(0, nc.sync)
            emit_w(1, nc.scalar)

            # Persistent accumulators: 4 mel banks + 2 shared cb banks.  Each
            # cb bank holds two t-blocks' [40, TB] outputs col-tiled into
            # partitions 0:40 and 64:104 (concurrent matmuls via tile_position).
            psm = [ps.tile([128, TB], f32, tag=f"m{t}", name=f"psm{t}") for t in range(NTB)]
            psc = [ps.tile([128, TB], f32, tag=f"c{p}", name=f"psc{p}") for p in range(NTB // 2)]

            # x chunk bookkeeping: kt -> (tile, col offset); chunk c covers
            # kts [cstart[c], cstart[c]+X_SCHED[c])
            x_of = {}
            cstart = []
            k = 0
            for xn in X_SCHED:
                cstart.append(k)
                k += xn

            def emit_x(c):
                xn = X_SCHED[c]
                k0 = cstart[c]
                ring = nc.sync if c % 2 == 0 else nc.scalar
                x_tile = xp.tile([128, xn * T], fin, tag="x", name=f"x{c}")
                if xn == 1:
                    ring.dma_start(x_tile[:], xt[k0 * 128:(k0 + 1) * 128, :])
                else:
                    ring.dma_start(
                        x_tile[:].rearrange("p (a t) -> p a t", a=xn),
                        xt[k0 * 128:(k0 + xn) * 128, :].rearrange(
                            "(a p) t -> p a t", p=128
                        ),
                    )
                for a in range(xn):
                    x_of[k0 + a] = (x_tile, a * T)

            def lhs(kt):
                wt, j = w_of[kt]
                return wt[:, j:j + 128], wt[:, j + 128:j + N_OUT]

            def rhs(kt, t):
                x_tile, off = x_of[kt]
                return x_tile[:, off + t * TB:off + (t + 1) * TB]

            def mel_mm(kt, t):
                nc.tensor.matmul(psm[t][:], lhs(kt)[0], rhs(kt, t),
                                 start=kt == 0, stop=kt == KT - 1)

            def cb_pair(kt, p):
                # two concurrent 40-col matmuls in distinct col groups
                lhs_c = lhs(kt)[1]
                first, last = kt == 0, kt == KT - 1
                nc.tensor.matmul(psc[p][0:40, :], lhs_c, rhs(kt, 2 * p),
                                 start=first, stop=last, tile_position=(0, 0))
                nc.tensor.matmul(psc[p][64:104, :], lhs_c, rhs(kt, 2 * p + 1),
                                 start=first, stop=last, tile_position=(0, 64))

            # Evictions: mel bank t -> out rows 0:128, cb bank p -> rows
            # 128:168 for t-blocks 2p/2p+1.  Copies split across vector (mel)
            # and gpsimd (cb) so the tail pipeline isn't serialized on DVE.
            def evict_mel(t):
                o1 = op.tile([128, TB], f32, tag=f"o{t}", name=f"o{t}")
                if t % 2 == 0:
                    nc.vector.tensor_copy(o1[:], psm[t][:])
                else:
                    nc.scalar.copy(o1[:], psm[t][:])
                ring = nc.sync if t % 2 == 0 else nc.scalar
                ring.dma_start(out[0:128, t * TB:(t + 1) * TB], o1[:])

            def evict_cb(p):
                o2 = op.tile([104, TB], f32, tag=f"oc{p}", name=f"oc{p}")
                if p % 2 == 0:
                    nc.vector.tensor_copy(o2[:], psc[p][0:104, :])
                else:
                    nc.scalar.copy(o2[:], psc[p][0:104, :])
                ring = nc.sync if p % 2 == 0 else nc.scalar
                ring.dma_start(out2[p], o2[:])

            for g, (k0, k1) in enumerate(GROUPS):
                # DMAs for this group's chunks: single-ktile chunks through
                # kt11 (fine-grained arrivals match the half-rate early PE),
                # then 1MB chunks; the W bulks ride after the last singles.
                if g < 3:
                    for c in range(4 * g, 4 * g + 4):
                        emit_x(c)
                    if g == 2:
                        emit_w(2, nc.sync)
                        emit_w(3, nc.scalar)
                else:
                    emit_x(2 * g + 6)
                    emit_x(2 * g + 7)

                if g < len(GROUPS) - 1:
                    # grouped steady state: all mel MMs back-to-back (LDWEIGHTS
                    # pipelined), then the cb pairs; 2 stationary-shape
                    # transitions per group instead of 2 per k-tile.
                    for kt in range(k0, k1):
                        for t in range(NTB):
                            mel_mm(kt, t)
                        # group 0: the supply of x chunks c2/c3 trails the PE
                        # by several us; DMA-independent warm matmuls bridge
                        # the gap at kt granularity so the PE stream stays
                        # continuous and the HAM clock grant isn't revoked.
                        if g == 0 and k0 + 1 <= kt <= k0 + 2:
                            warm_mm(4)
                    for kt in range(k0, k1):
                        cb_pair(kt, 0)
                        cb_pair(kt, 1)
                    # early groups: DMA-independent filler matmuls keep the PE
                    # active through any x-starvation gap so the HAM
                    # utilization monitor grants full clock ASAP.
                    if g < 2:
                        warm_mm(1)
                else:
                    # final group: bank-major order so evictions start while
                    # the remaining banks' matmuls still run; cb first so the
                    # kernel ends on the cheap single-DMA mel stores.
                    for p in range(NTB // 2):
                        for kt in range(k0, k1):
                            cb_pair(kt, p)
                        evict_cb(p)
                    for t in range(NTB):
                        for kt in range(k0, k1):
                            mel_mm(kt, t)
                        evict_mel(t)
    return nc


def _get_nc():
    global _NC
    if _NC is None:
        _NC = _build_nc()
        if not _NC.is_finalized():
            _NC.finalize()
    return _NC


def kernel(**inputs):
    global LAST_RESULT
    melody = np.ascontiguousarray(np.asarray(inputs["melody_tensor"], dtype=np.float32))
    lyrics = np.ascontiguousarray(np.asarray(inputs["lyrics_tensor"], dtype=np.float32))
    emb = np.asarray(inputs["emb"], dtype=np.float32)
    conv_w = np.asarray(inputs["conv_w"], dtype=np.float32)
    conv_b = np.asarray(inputs["conv_b"], dtype=np.float32)
    w_chord = np.asarray(inputs["w_chord"], dtype=np.float32)
    w_beat = np.asarray(inputs["w_beat"], dtype=np.float32)
    w_mel = np.asarray(inputs["w_mel"], dtype=np.float32)
    b_heads = np.concatenate([
        np.asarray(inputs["b_chord"], dtype=np.float32),
        np.asarray(inputs["b_beat"], dtype=np.float32),
        np.asarray(inputs["b_mel"], dtype=np.float32),
    ])
    genre = int(np.asarray(inputs["genre"]).reshape(-1)[0])
    tempo = int(np.asarray(inputs["tempo"]).reshape(-1)[0])
    key_sig = int(np.asarray(inputs["key_sig"]).reshape(-1)[0])

    # Fold conv into head weights: W'[e] = k0*W[e+1] + k1*W[e] + k2*W[e-1]
    W = np.concatenate([w_chord, w_beat, w_mel], axis=1)  # [50937, 168]
    k0, k1, k2 = (float(v) for v in conv_w[0, 0, 1, :])
    Wp = k1 * W
    Wp[:-1] += k0 * W[1:]
    Wp[1:] += k2 * W[:-1]

    # Bias: head biases + conv bias * colsum(W) + context-embedding term
    ids = [genre, 10 + tempo, 20 + key_sig, 34]
    ctx = emb[ids].sum(axis=0).astype(np.float64)  # [256]
    bias = (
        b_heads.astype(np.float64)
        + float(conv_b[0]) * W.sum(axis=0, dtype=np.float64)
        + ctx @ Wp[0:256].astype(np.float64)
    )  # [168]

    # Device operands: xT [51200, 2048] (zero padded), W' rows 256.. packed
    np_dt = _np_in_dt()
    K_PAD = N_CORES * K_PER
    XT = np.zeros((K_PAD, T), np_dt)
    XT[0:256] = melody.T
    XT[256:K_GEMM] = lyrics.T
    Wg = np.zeros((K_PAD, N_OUT), np_dt)
    Wg[0:K_GEMM] = Wp[256:]

    in_maps = []
    for c in range(N_CORES):
        wc = (
            Wg[c * K_PER:(c + 1) * K_PER]
            .reshape(KT, 128, N_OUT)
            .transpose(1, 0, 2)
            .reshape(128, KT * N_OUT)
        )
        in_maps.append({
            "xt": XT[c * K_PER:(c + 1) * K_PER],
            "w": np.ascontiguousarray(wc),
        })

    trace = bool(os.environ.get("HARMONY_TRACE"))
    res = run_bass_kernel_spmd(_get_nc(), in_maps, core_ids=list(range(N_CORES)), trace=trace)
    LAST_RESULT = res

    acc = np.zeros((N_OUT, T), np.float64)
    for r in res.results:
        acc[0:128] += r["out"]
        o2 = r["out2"]
        for p in range(2):
            acc[128:N_OUT, 2 * p * TB:(2 * p + 1) * TB] += o2[p][0:40]
            acc[128:N_OUT, (2 * p + 1) * TB:(2 * p + 2) * TB] += o2[p][64:104]
    out = (acc + bias[:, None]).T
    return np.ascontiguousarray(out.astype(np.float32))
